# revision 7
# baseline (speedup 1.0000x reference)
"""MixedArityTreeLSTM Trainium2 kernel (v2: pair-split layout).

Level-synchronous bottom-up Tree-LSTM over B=256 heap-indexed perfect binary
trees (511 nodes, depth 8), E=H=128. Pure data-parallel over 8 NeuronCores
(32 trees per core); all weights replicated.

Per-core layout: activations feature-major [H(part), nodes(free)]. Each
level is stored in "pair-split" order: for the parent level's column order
sigma_l, the child level is stored as [all left children in sigma_l order |
all right children in sigma_l order]. This makes every child operand a
CONTIGUOUS block (left block / right block), so vector ops run in 2x DVE
mode and no strided views are needed. The permutations are baked host-side
into the gather indices and masks; level 0 order is the identity.

Arity blending is folded into the matmuls via masked children + Uun-folding:
    pre_g = W_g^T x + Uun_g^T h_l + (Ubt_g - Uun_g)^T (m*h_l)
            + Ubb_g^T (m*h_r) + m*(b_bin_g - b_un_g) [K=1 matmul]
            + (bW_g + b_un_g) [ACT bias]
Matmul operands bf16; PSUM fp32 (leaf psum bf16); gates/h/c stored bf16.
Embedding gather uses gpsimd dma_gather(transpose=True) on a bf16 table.
"""

import os

import numpy as np
import ml_dtypes

DBG_MIN_LVL = int(os.environ.get("TL_MIN_LVL", "0"))
N_QUEUES = int(os.environ.get("TL_NQ", "2"))
C_FP32 = os.environ.get("TL_C_FP32", "") == "1"  # keep c/gates fp32 (debug)

B, D = 256, 8
V, E, H = 32000, 128, 128
N_NODES = 2 ** (D + 1) - 1  # 511
NCORES = 8
BL = B // NCORES  # 32 trees per core

LVL_N = {l: BL * (2**l) for l in range(D + 1)}
LVL_PW = {l: max(128, BL * (2**l)) for l in range(D + 1)}

# chunk widths: leaves at 1024, internal levels at 512
CHUNKW = {l: 512 for l in range(D + 1)}
CPL = {l: max(1, LVL_N[l] // CHUNKW[l]) for l in range(D + 1)}

# pair-split sigma orders (within-level tree-major flat indices)
SIG = {0: np.arange(BL, dtype=np.int64)}
for _l in range(0, D):
    _f = SIG[_l]
    _t, _p = _f // (2**_l), _f % (2**_l)
    _le = _t * (2 ** (_l + 1)) + 2 * _p
    SIG[_l + 1] = np.concatenate([_le, _le + 1])


def _child_chunks(l, j):
    """Child-level chunk ids needed by compute chunk (l, j)."""
    c0 = j * CHUNKW[l]
    N = min(CHUNKW[l], LVL_N[l] - c0)
    half = LVL_N[l]
    cw = CHUNKW[l + 1]
    s = set()
    for a, b in ((c0, c0 + N), (half + c0, half + c0 + N)):
        for k in range(a // cw, (b - 1) // cw + 1):
            s.add(k)
    return sorted(s)


ORDER = []
_emitted = set()


def _post(l, j):
    if (l, j) in _emitted:
        return
    if l < D:
        for k in _child_chunks(l, j):
            _post(l + 1, k)
    _emitted.add((l, j))
    ORDER.append((l, j))


_post(0, 0)

# gather calls, one per chunk in wave order: (lvl, col0, width in padded xT)
GATHER_CALLS = [
    (lvl, j * CHUNKW[lvl], min(CHUNKW[lvl], LVL_PW[lvl] - j * CHUNKW[lvl]))
    for lvl, j in ORDER
]

# internal-level compute chunks in wave order: (cid, lvl, c0, N, mask offset)
CHUNKS = []
_moff = 0
for lvl, j in ORDER:
    if lvl == D:
        continue
    N = min(CHUNKW[lvl], LVL_N[lvl] - j * CHUNKW[lvl])
    CHUNKS.append((len(CHUNKS), lvl, j * CHUNKW[lvl], N, _moff))
    _moff += N
MASKB_LEN = _moff  # 8160

IDX_COLS = sum(w // 16 for _, _, w in GATHER_CALLS)

BF16 = ml_dtypes.bfloat16

_CACHE = {}


def _build_nc():
    if "nc" in _CACHE:
        return _CACHE["nc"]

    from contextlib import ExitStack

    import concourse.mybir as mybir
    import concourse.tile as tile
    from concourse import bacc

    dt = mybir.dt
    AF = mybir.ActivationFunctionType
    gdt = dt.float32 if C_FP32 else dt.bfloat16

    nc = bacc.Bacc(num_swdge_queues=N_QUEUES)

    emb_d = nc.dram_tensor("emb_bf", [V, E], dt.bfloat16, kind="ExternalInput")
    idx_d = nc.dram_tensor("gidx", [128, IDX_COLS], dt.int16, kind="ExternalInput")
    mbc_d = nc.dram_tensor(
        "mbcast", [128, MASKB_LEN], dt.bfloat16, kind="ExternalInput"
    )
    maskb_d = nc.dram_tensor(
        "maskb", [1, MASKB_LEN], dt.bfloat16, kind="ExternalInput"
    )
    w_d = nc.dram_tensor("w_bf", [4, E, H], dt.bfloat16, kind="ExternalInput")
    # ubt_eff rows: i,fl: Ubt-Uun; fr: Ubt; o,u: Ubt-Uun (see prep)
    ubt_d = nc.dram_tensor("ubt_bf", [5, H, H], dt.bfloat16, kind="ExternalInput")
    ubb_d = nc.dram_tensor("ubb_bf", [5, H, H], dt.bfloat16, kind="ExternalInput")
    uun_d = nc.dram_tensor("uun_bf", [4, H, H], dt.bfloat16, kind="ExternalInput")
    # bias rows: 0=b_leaf 1=bc_i 2=bc_fL 3=b_fR 4=bc_o 5=bc_u
    bias_d = nc.dram_tensor("biases", [6, H], dt.float32, kind="ExternalInput")
    # delta rows: 0=d_i 1=d_fL 2=d_o 3=d_u 4=+40 (f_r unary kill)
    delt_d = nc.dram_tensor("deltas", [5, H], dt.bfloat16, kind="ExternalInput")

    h_out_d = nc.dram_tensor("h_out", [H, BL], dt.float32, kind="ExternalOutput")
    c_out_d = nc.dram_tensor("c_out", [H, BL], dt.float32, kind="ExternalOutput")

    with tile.TileContext(nc) as tc, ExitStack() as ctx:
        consts = ctx.enter_context(tc.tile_pool(name="consts", bufs=1))

        w_sb = consts.tile([E, 4, H], dt.bfloat16)
        nc.sync.dma_start(out=w_sb, in_=w_d[:, :, :].rearrange("g e h -> e g h"))
        ubt_sb = consts.tile([H, 5, H], dt.bfloat16)
        nc.sync.dma_start(out=ubt_sb, in_=ubt_d[:, :, :].rearrange("g k h -> k g h"))
        ubb_sb = consts.tile([H, 5, H], dt.bfloat16)
        nc.sync.dma_start(out=ubb_sb, in_=ubb_d[:, :, :].rearrange("g k h -> k g h"))
        uun_sb = consts.tile([H, 4, H], dt.bfloat16)
        nc.sync.dma_start(out=uun_sb, in_=uun_d[:, :, :].rearrange("g k h -> k g h"))
        bias_sb = consts.tile([H, 6], dt.float32)
        nc.sync.dma_start(out=bias_sb, in_=bias_d[:, :].rearrange("n h -> h n"))
        delt_sb = consts.tile([1, 5, H], dt.bfloat16)
        nc.sync.dma_start(
            out=delt_sb, in_=delt_d[:, :].rearrange("(o g) h -> o g h", o=1)
        )
        idx_sb = consts.tile([128, IDX_COLS], dt.int16)
        nc.sync.dma_start(out=idx_sb, in_=idx_d[:, :])
        mbc_sb = consts.tile([128, MASKB_LEN], dt.bfloat16)
        nc.sync.dma_start(out=mbc_sb, in_=mbc_d[:, :])
        maskb_sb = consts.tile([1, MASKB_LEN], dt.bfloat16)
        nc.sync.dma_start(out=maskb_sb, in_=maskb_d[:, :])

        # per-level xT tiles
        lev = ctx.enter_context(tc.tile_pool(name="lev", bufs=1))
        xt = {}
        for lvl in range(D, -1, -1):
            xt[lvl] = lev.tile(
                [128, LVL_PW[lvl]], dt.bfloat16, name=f"xTl{lvl}", tag=f"xTl{lvl}"
            )

        psum = ctx.enter_context(tc.tile_pool(name="psum", bufs=6, space="PSUM"))
        psumL = ctx.enter_context(tc.tile_pool(name="psumL", bufs=2, space="PSUM"))
        work = ctx.enter_context(tc.tile_pool(name="work", bufs=2))

        h_t = {}
        c_t = {}
        h_t[D] = lev.tile([H, LVL_N[D]], dt.bfloat16, name="h_leaf", tag="h_leaf")

        icols = {}
        _ic = 0
        for gi_, (lvl, c0, width) in enumerate(GATHER_CALLS):
            icols[(lvl, c0)] = (_ic, width, gi_)
            _ic += width // 16

        cid_of = {(lvl, c0): (cid, N, moff) for cid, lvl, c0, N, moff in CHUNKS}

        # gate -> (W idx, ubt_eff idx, uun idx or None, delta idx, bias col)
        GATES_FULL = [
            ("i", 0, 0, 0, 0, 1),
            ("fl", 1, 1, 1, 1, 2),
            ("fr", 1, 2, None, 4, 3),
            ("o", 2, 3, 2, 2, 4),
            ("u", 3, 4, 3, 3, 5),
        ]
        GATES_TOP = [GATES_FULL[0], GATES_FULL[3], GATES_FULL[4]]

        for lvl, j in ORDER:
            g0 = j * CHUNKW[lvl]
            _icol, width, gi_ = icols[(lvl, g0)]
            out_view = xt[lvl][:, g0 : g0 + width].rearrange(
                "p (o n) -> p o n", o=1
            )
            nc.gpsimd.dma_gather(
                out_view,
                emb_d[:, :],
                idx_sb[:, _icol : _icol + width // 16],
                width,
                width,
                E,
                transpose=True,
                queue_num=gi_ % N_QUEUES,
            )

            if lvl == D:
                for s0 in range(g0, g0 + width, 512):
                    sw = min(512, g0 + width - s0)
                    ps = psumL.tile([H, sw], dt.float32, tag="psL", name="ps_leaf")
                    nc.tensor.matmul(
                        ps, w_sb[:, 3, :], xt[D][:, s0 : s0 + sw],
                        start=True, stop=True,
                    )
                    nc.scalar.activation(
                        h_t[D][:, s0 : s0 + sw], ps, AF.Tanh, bias=bias_sb[:, 0:1]
                    )
                continue

            cid, N, moff = cid_of[(lvl, g0)]
            c0 = g0
            if lvl < DBG_MIN_LVL:
                continue
            if c0 == 0:
                n = LVL_N[lvl]
                hdt = dt.float32 if lvl == 0 else dt.bfloat16
                cdt = dt.float32 if lvl == 0 else gdt
                h_t[lvl] = lev.tile([H, n], hdt, name=f"h_l{lvl}", tag=f"h_l{lvl}")
                c_t[lvl] = lev.tile([H, n], cdt, name=f"c_l{lvl}", tag=f"c_l{lvl}")

            half = LVL_N[lvl]
            hch = h_t[lvl + 1]
            h_l = hch[:, c0 : c0 + N]
            h_r = hch[:, half + c0 : half + c0 + N]

            mb = mbc_sb[:, moff : moff + N]
            hlb = work.tile([128, N], dt.bfloat16, tag="hlb", name="hlb")
            nc.vector.tensor_mul(hlb, h_l, mb)
            hrb = work.tile([128, N], dt.bfloat16, tag="hrb", name="hrb")
            nc.vector.tensor_mul(hrb, h_r, mb)

            xs = xt[lvl][:, c0 : c0 + N]
            mrow = maskb_sb[:, moff : moff + N]
            top = lvl == D - 1
            gates = GATES_TOP if top else GATES_FULL

            pts = {}
            for gname, wi, ubi, uui, di, _bi in gates:
                ps = psum.tile([H, N], dt.float32, tag="pg", name=f"ps_{gname}")
                # child-independent matmuls first: PE can start while DVE
                # still produces hlb/hrb
                nc.tensor.matmul(
                    ps, delt_sb[:, di, :], mrow, start=True, stop=False
                )
                nc.tensor.matmul(ps, w_sb[:, wi, :], xs, start=False, stop=False)
                if uui is not None:
                    nc.tensor.matmul(
                        ps, uun_sb[:, uui, :], h_l, start=False, stop=False
                    )
                nc.tensor.matmul(ps, ubt_sb[:, ubi, :], hlb, start=False, stop=False)
                nc.tensor.matmul(
                    ps, ubb_sb[:, ubi, :], hrb, start=False, stop=True
                )
                pts[gname] = ps

            gtiles = {}
            for gname, _wi, _ubi, _uui, _di, bi in gates:
                g = work.tile([128, N], gdt, tag=f"g{gname}", name=f"g{gname}")
                fn = AF.Tanh if gname == "u" else AF.Sigmoid
                nc.scalar.activation(g, pts[gname], fn, bias=bias_sb[:, bi : bi + 1])
                gtiles[gname] = g

            cs = c_t[lvl][:, c0 : c0 + N]
            if top:
                nc.vector.tensor_mul(cs, gtiles["i"], gtiles["u"])
            else:
                cch = c_t[lvl + 1]
                c_l = cch[:, c0 : c0 + N]
                c_r = cch[:, half + c0 : half + c0 + N]
                t1 = work.tile([128, N], gdt, tag="t1", name="t1")
                nc.vector.tensor_mul(t1, gtiles["i"], gtiles["u"])
                f2l = work.tile([128, N], gdt, tag="f2l", name="f2l")
                nc.vector.tensor_mul(f2l, gtiles["fl"], c_l)
                f2r = work.tile([128, N], gdt, tag="f2r", name="f2r")
                nc.vector.tensor_mul(f2r, gtiles["fr"], c_r)
                fs = work.tile([128, N], gdt, tag="fs", name="fs")
                nc.vector.tensor_add(fs, f2l, f2r)
                nc.vector.tensor_add(cs, t1, fs)

            tch = work.tile([128, N], gdt, tag="tch", name="tch")
            nc.scalar.activation(tch, cs, AF.Tanh)
            nc.vector.tensor_mul(h_t[lvl][:, c0 : c0 + N], gtiles["o"], tch)

        ol = DBG_MIN_LVL
        h_fin = h_t[ol][:, :BL]
        c_fin = c_t[ol][:, :BL] if ol in c_t else h_t[ol][:, :BL]
        eng = nc.sync if ol == 0 else nc.gpsimd
        eng.dma_start(out=h_out_d[:, :], in_=h_fin)
        eng.dma_start(out=c_out_d[:, :], in_=c_fin)

    nc.finalize()
    _CACHE["nc"] = nc
    return nc


def _wrap_idx(seg):
    """dma_gather index layout: unwrapped[i] = idxs[i % 16, i // 16],
    replicated across the 128 partitions."""
    w = seg.reshape(-1, 16).T.astype(np.int16)
    return np.tile(w, (8, 1))


def prep_core_inputs(tokens_c, arity_c, shared):
    """Per-core input map. tokens_c [BL,511], arity_c [BL,255]."""
    tokens_c = np.asarray(tokens_c)
    arity_c = np.asarray(arity_c)

    # per-level sigma-ordered tokens (padded to LVL_PW)
    tok_sig = {}
    for l in range(D + 1):
        off, cnt = 2**l - 1, 2**l
        flat = tokens_c[:, off : off + cnt].reshape(-1)[SIG[l]]
        pw = LVL_PW[l]
        if pw != flat.size:
            flat = np.concatenate([flat, np.zeros(pw - flat.size, np.int64)])
        tok_sig[l] = flat

    idx_cols = []
    for lvl, c0, width in GATHER_CALLS:
        idx_cols.append(_wrap_idx(tok_sig[lvl][c0 : c0 + width]))
    gidx = np.concatenate(idx_cols, axis=1)
    assert gidx.shape == (128, IDX_COLS)

    maskb = np.zeros((1, MASKB_LEN), BF16)
    for cid, lvl, c0, N, moff in CHUNKS:
        off = 2**lvl - 1
        m_flat = (arity_c[:, off : off + 2**lvl].reshape(-1) == 1).astype(
            np.float32
        )[SIG[lvl]]
        maskb[0, moff : moff + N] = m_flat[c0 : c0 + N].astype(BF16)

    return dict(
        shared,
        gidx=gidx,
        maskb=maskb,
        mbcast=np.broadcast_to(maskb, (128, MASKB_LEN)).copy(),
    )


def prep_shared_inputs(emb, W, bW, Ubin, bUbin, Uun, bUun):
    emb = np.asarray(emb, np.float32)
    W = np.asarray(W, np.float32)
    bW = np.asarray(bW, np.float32)
    Ubin = np.asarray(Ubin, np.float32)
    bUbin = np.asarray(bUbin, np.float32)
    Uun = np.asarray(Uun, np.float32)
    bUun = np.asarray(bUun, np.float32)

    biases = np.stack(
        [
            bW[3],                # leaf
            bW[0] + bUun[0],      # i common
            bW[1] + bUun[1],      # fL common
            bW[1] + bUbin[2] - 40.0,  # fR (binary-only; -40 kills unary)
            bW[2] + bUun[2],      # o common
            bW[3] + bUun[3],      # u common
        ]
    ).astype(np.float32)
    deltas = np.stack(
        [
            bUbin[0] - bUun[0],
            bUbin[1] - bUun[1],
            bUbin[3] - bUun[2],
            bUbin[4] - bUun[3],
            np.full(H, 40.0, np.float32),
        ]
    ).astype(BF16)

    ubt = Ubin[:, :H, :].copy()  # top half acts on h_l
    # Uun-folding: gates i,fl,o,u get (Ubt - Uun); fr keeps Ubt
    ubt_eff = ubt.copy()
    ubt_eff[0] -= Uun[0]
    ubt_eff[1] -= Uun[1]
    ubt_eff[3] -= Uun[2]
    ubt_eff[4] -= Uun[3]

    return dict(
        emb_bf=emb.astype(BF16),
        w_bf=W.astype(BF16),
        ubt_bf=ubt_eff.astype(BF16),
        ubb_bf=Ubin[:, H:, :].astype(BF16),
        uun_bf=Uun.astype(BF16),
        biases=biases,
        deltas=deltas,
    )


def kernel(tokens, arity, emb, W, bW, Ubin, bUbin, Uun, bUun):
    from concourse.bass_utils import run_bass_kernel_spmd

    tokens = np.asarray(tokens)
    arity = np.asarray(arity)

    shared = prep_shared_inputs(emb, W, bW, Ubin, bUbin, Uun, bUun)
    in_maps = [
        prep_core_inputs(
            tokens[k * BL : (k + 1) * BL], arity[k * BL : (k + 1) * BL], shared
        )
        for k in range(NCORES)
    ]

    nc = _build_nc()
    res = run_bass_kernel_spmd(nc, in_maps, core_ids=list(range(NCORES)))
    results = res.results

    h = np.concatenate([r["h_out"].T for r in results], axis=0)
    c = np.concatenate([r["c_out"].T for r in results], axis=0)
    return h.astype(np.float32), c.astype(np.float32)


# revision 8
# speedup vs baseline: 1.2857x; 1.2857x over previous
"""MixedArityTreeLSTM Trainium2 kernel (v2: pair-split layout).

Level-synchronous bottom-up Tree-LSTM over B=256 heap-indexed perfect binary
trees (511 nodes, depth 8), E=H=128. Pure data-parallel over 8 NeuronCores
(32 trees per core); all weights replicated.

Per-core layout: activations feature-major [H(part), nodes(free)]. Each
level is stored in "pair-split" order: for the parent level's column order
sigma_l, the child level is stored as [all left children in sigma_l order |
all right children in sigma_l order]. This makes every child operand a
CONTIGUOUS block (left block / right block), so vector ops run in 2x DVE
mode and no strided views are needed. The permutations are baked host-side
into the gather indices and masks; level 0 order is the identity.

Arity blending is folded into the matmuls via masked children + Uun-folding:
    pre_g = W_g^T x + Uun_g^T h_l + (Ubt_g - Uun_g)^T (m*h_l)
            + Ubb_g^T (m*h_r) + m*(b_bin_g - b_un_g) [K=1 matmul]
            + (bW_g + b_un_g) [ACT bias]
Matmul operands bf16; PSUM fp32 (leaf psum bf16); gates/h/c stored bf16.
Embedding gather uses gpsimd dma_gather(transpose=True) on a bf16 table.
"""

import os

import numpy as np
import ml_dtypes

DBG_MIN_LVL = int(os.environ.get("TL_MIN_LVL", "0"))
N_QUEUES = int(os.environ.get("TL_NQ", "2"))
C_FP32 = os.environ.get("TL_C_FP32", "") == "1"  # keep c/gates fp32 (debug)

B, D = 256, 8
V, E, H = 32000, 128, 128
N_NODES = 2 ** (D + 1) - 1  # 511
NCORES = 8
BL = B // NCORES  # 32 trees per core

LVL_N = {l: BL * (2**l) for l in range(D + 1)}
LVL_PW = {l: max(128, BL * (2**l)) for l in range(D + 1)}

# chunk widths: leaves at 1024, internal levels at 512
CHUNKW = {l: 512 for l in range(D + 1)}
CPL = {l: max(1, LVL_N[l] // CHUNKW[l]) for l in range(D + 1)}

# pair-split sigma orders (within-level tree-major flat indices)
SIG = {0: np.arange(BL, dtype=np.int64)}
for _l in range(0, D):
    _f = SIG[_l]
    _t, _p = _f // (2**_l), _f % (2**_l)
    _le = _t * (2 ** (_l + 1)) + 2 * _p
    SIG[_l + 1] = np.concatenate([_le, _le + 1])


def _child_chunks(l, j):
    """Child-level chunk ids needed by compute chunk (l, j)."""
    c0 = j * CHUNKW[l]
    N = min(CHUNKW[l], LVL_N[l] - c0)
    half = LVL_N[l]
    cw = CHUNKW[l + 1]
    s = set()
    for a, b in ((c0, c0 + N), (half + c0, half + c0 + N)):
        for k in range(a // cw, (b - 1) // cw + 1):
            s.add(k)
    return sorted(s)


ORDER = []
_emitted = set()


def _post(l, j):
    if (l, j) in _emitted:
        return
    if l < D:
        for k in _child_chunks(l, j):
            _post(l + 1, k)
    _emitted.add((l, j))
    ORDER.append((l, j))


_post(0, 0)

# gather calls, one per chunk in wave order: (lvl, col0, width in padded xT)
GATHER_CALLS = [
    (lvl, j * CHUNKW[lvl], min(CHUNKW[lvl], LVL_PW[lvl] - j * CHUNKW[lvl]))
    for lvl, j in ORDER
]

# internal-level compute chunks in wave order: (cid, lvl, c0, N, mask offset)
CHUNKS = []
_moff = 0
for lvl, j in ORDER:
    if lvl == D:
        continue
    N = min(CHUNKW[lvl], LVL_N[lvl] - j * CHUNKW[lvl])
    CHUNKS.append((len(CHUNKS), lvl, j * CHUNKW[lvl], N, _moff))
    _moff += N
MASKB_LEN = _moff  # 8160

IDX_COLS = sum(w // 16 for _, _, w in GATHER_CALLS)

BF16 = ml_dtypes.bfloat16

_CACHE = {}


def _build_nc():
    if "nc" in _CACHE:
        return _CACHE["nc"]

    from contextlib import ExitStack

    import concourse.mybir as mybir
    import concourse.tile as tile
    from concourse import bacc

    dt = mybir.dt
    AF = mybir.ActivationFunctionType
    gdt = dt.float32 if C_FP32 else dt.bfloat16

    nc = bacc.Bacc(num_swdge_queues=N_QUEUES)

    emb_d = nc.dram_tensor("emb_bf", [V, E], dt.bfloat16, kind="ExternalInput")
    idx_d = nc.dram_tensor("gidx", [128, IDX_COLS], dt.int16, kind="ExternalInput")
    mbc_d = nc.dram_tensor(
        "mbcast", [128, MASKB_LEN], dt.bfloat16, kind="ExternalInput"
    )
    maskb_d = nc.dram_tensor(
        "maskb", [1, MASKB_LEN], dt.bfloat16, kind="ExternalInput"
    )
    w_d = nc.dram_tensor("w_bf", [4, E, H], dt.bfloat16, kind="ExternalInput")
    # ubt_eff rows: i,fl: Ubt-Uun; fr: Ubt; o,u: Ubt-Uun (see prep)
    ubt_d = nc.dram_tensor("ubt_bf", [5, H, H], dt.bfloat16, kind="ExternalInput")
    ubb_d = nc.dram_tensor("ubb_bf", [5, H, H], dt.bfloat16, kind="ExternalInput")
    uun_d = nc.dram_tensor("uun_bf", [4, H, H], dt.bfloat16, kind="ExternalInput")
    # bias rows: 0=b_leaf 1=bc_i 2=bc_fL 3=b_fR 4=bc_o 5=bc_u
    bias_d = nc.dram_tensor("biases", [6, H], dt.float32, kind="ExternalInput")
    # delta rows: 0=d_i 1=d_fL 2=d_o 3=d_u 4=+40 (f_r unary kill)
    delt_d = nc.dram_tensor("deltas", [5, H], dt.bfloat16, kind="ExternalInput")

    h_out_d = nc.dram_tensor("h_out", [H, BL], dt.float32, kind="ExternalOutput")
    c_out_d = nc.dram_tensor("c_out", [H, BL], dt.float32, kind="ExternalOutput")

    with tile.TileContext(nc) as tc, ExitStack() as ctx:
        consts = ctx.enter_context(tc.tile_pool(name="consts", bufs=1))

        w_sb = consts.tile([E, 4, H], dt.bfloat16)
        nc.sync.dma_start(out=w_sb, in_=w_d[:, :, :].rearrange("g e h -> e g h"))
        ubt_sb = consts.tile([H, 5, H], dt.bfloat16)
        nc.sync.dma_start(out=ubt_sb, in_=ubt_d[:, :, :].rearrange("g k h -> k g h"))
        ubb_sb = consts.tile([H, 5, H], dt.bfloat16)
        nc.sync.dma_start(out=ubb_sb, in_=ubb_d[:, :, :].rearrange("g k h -> k g h"))
        uun_sb = consts.tile([H, 4, H], dt.bfloat16)
        nc.sync.dma_start(out=uun_sb, in_=uun_d[:, :, :].rearrange("g k h -> k g h"))
        bias_sb = consts.tile([H, 6], dt.float32)
        nc.sync.dma_start(out=bias_sb, in_=bias_d[:, :].rearrange("n h -> h n"))
        delt_sb = consts.tile([1, 5, H], dt.bfloat16)
        nc.sync.dma_start(
            out=delt_sb, in_=delt_d[:, :].rearrange("(o g) h -> o g h", o=1)
        )
        idx_sb = consts.tile([128, IDX_COLS], dt.int16)
        nc.sync.dma_start(out=idx_sb, in_=idx_d[:, :])
        mbc_sb = consts.tile([128, MASKB_LEN], dt.bfloat16)
        nc.sync.dma_start(out=mbc_sb, in_=mbc_d[:, :])
        maskb_sb = consts.tile([1, MASKB_LEN], dt.bfloat16)
        nc.sync.dma_start(out=maskb_sb, in_=maskb_d[:, :])

        # per-level xT tiles
        lev = ctx.enter_context(tc.tile_pool(name="lev", bufs=1))
        xt = {}
        for lvl in range(D, -1, -1):
            xt[lvl] = lev.tile(
                [128, LVL_PW[lvl]], dt.bfloat16, name=f"xTl{lvl}", tag=f"xTl{lvl}"
            )

        psum = ctx.enter_context(tc.tile_pool(name="psum", bufs=8, space="PSUM"))
        work = ctx.enter_context(tc.tile_pool(name="work", bufs=2))

        h_t = {}
        c_t = {}
        h_t[D] = lev.tile([H, LVL_N[D]], dt.bfloat16, name="h_leaf", tag="h_leaf")

        icols = {}
        _ic = 0
        for gi_, (lvl, c0, width) in enumerate(GATHER_CALLS):
            icols[(lvl, c0)] = (_ic, width, gi_)
            _ic += width // 16

        cid_of = {(lvl, c0): (cid, N, moff) for cid, lvl, c0, N, moff in CHUNKS}

        # gate -> (W idx, ubt_eff idx, uun idx or None, delta idx, bias col)
        GATES_FULL = [
            ("i", 0, 0, 0, 0, 1),
            ("fl", 1, 1, 1, 1, 2),
            ("fr", 1, 2, None, 4, 3),
            ("o", 2, 3, 2, 2, 4),
            ("u", 3, 4, 3, 3, 5),
        ]
        GATES_TOP = [GATES_FULL[0], GATES_FULL[3], GATES_FULL[4]]

        for lvl, j in ORDER:
            g0 = j * CHUNKW[lvl]
            _icol, width, gi_ = icols[(lvl, g0)]
            out_view = xt[lvl][:, g0 : g0 + width].rearrange(
                "p (o n) -> p o n", o=1
            )
            nc.gpsimd.dma_gather(
                out_view,
                emb_d[:, :],
                idx_sb[:, _icol : _icol + width // 16],
                width,
                width,
                E,
                transpose=True,
                queue_num=gi_ % N_QUEUES,
            )

            if lvl == D:
                for s0 in range(g0, g0 + width, 512):
                    sw = min(512, g0 + width - s0)
                    ps = psum.tile([H, sw], dt.float32, tag="pg", name="ps_leaf")
                    nc.tensor.matmul(
                        ps, w_sb[:, 3, :], xt[D][:, s0 : s0 + sw],
                        start=True, stop=True,
                    )
                    nc.scalar.activation(
                        h_t[D][:, s0 : s0 + sw], ps, AF.Tanh, bias=bias_sb[:, 0:1]
                    )
                continue

            cid, N, moff = cid_of[(lvl, g0)]
            c0 = g0
            if lvl < DBG_MIN_LVL:
                continue
            if c0 == 0:
                n = LVL_N[lvl]
                hdt = dt.float32 if lvl == 0 else dt.bfloat16
                cdt = dt.float32 if lvl == 0 else gdt
                h_t[lvl] = lev.tile([H, n], hdt, name=f"h_l{lvl}", tag=f"h_l{lvl}")
                c_t[lvl] = lev.tile([H, n], cdt, name=f"c_l{lvl}", tag=f"c_l{lvl}")

            half = LVL_N[lvl]
            hch = h_t[lvl + 1]
            h_l = hch[:, c0 : c0 + N]
            h_r = hch[:, half + c0 : half + c0 + N]

            mb = mbc_sb[:, moff : moff + N]
            hlb = work.tile([128, N], dt.bfloat16, tag="hlb", name="hlb")
            nc.vector.tensor_mul(hlb, h_l, mb)
            hrb = work.tile([128, N], dt.bfloat16, tag="hrb", name="hrb")
            nc.vector.tensor_mul(hrb, h_r, mb)

            xs = xt[lvl][:, c0 : c0 + N]
            mrow = maskb_sb[:, moff : moff + N]
            top = lvl == D - 1
            gates = GATES_TOP if top else GATES_FULL

            pts = {}
            for gname, wi, ubi, uui, di, _bi in gates:
                ps = psum.tile([H, N], dt.float32, tag="pg", name=f"ps_{gname}")
                # child-independent matmuls first: PE can start while DVE
                # still produces hlb/hrb
                nc.tensor.matmul(
                    ps, delt_sb[:, di, :], mrow, start=True, stop=False
                )
                nc.tensor.matmul(ps, w_sb[:, wi, :], xs, start=False, stop=False)
                if uui is not None:
                    nc.tensor.matmul(
                        ps, uun_sb[:, uui, :], h_l, start=False, stop=False
                    )
                nc.tensor.matmul(ps, ubt_sb[:, ubi, :], hlb, start=False, stop=False)
                nc.tensor.matmul(
                    ps, ubb_sb[:, ubi, :], hrb, start=False, stop=True
                )
                pts[gname] = ps

            gtiles = {}
            for gname, _wi, _ubi, _uui, _di, bi in gates:
                g = work.tile([128, N], gdt, tag=f"g{gname}", name=f"g{gname}")
                fn = AF.Tanh if gname == "u" else AF.Sigmoid
                nc.scalar.activation(g, pts[gname], fn, bias=bias_sb[:, bi : bi + 1])
                gtiles[gname] = g

            cs = c_t[lvl][:, c0 : c0 + N]
            if top:
                nc.vector.tensor_mul(cs, gtiles["i"], gtiles["u"])
            else:
                cch = c_t[lvl + 1]
                c_l = cch[:, c0 : c0 + N]
                c_r = cch[:, half + c0 : half + c0 + N]
                t1 = work.tile([128, N], gdt, tag="t1", name="t1")
                nc.vector.tensor_mul(t1, gtiles["i"], gtiles["u"])
                f2l = work.tile([128, N], gdt, tag="f2l", name="f2l")
                nc.vector.tensor_mul(f2l, gtiles["fl"], c_l)
                f2r = work.tile([128, N], gdt, tag="f2r", name="f2r")
                nc.vector.tensor_mul(f2r, gtiles["fr"], c_r)
                fs = work.tile([128, N], gdt, tag="fs", name="fs")
                nc.vector.tensor_add(fs, f2l, f2r)
                nc.vector.tensor_add(cs, t1, fs)

            tch = work.tile([128, N], gdt, tag="tch", name="tch")
            nc.scalar.activation(tch, cs, AF.Tanh)
            nc.vector.tensor_mul(h_t[lvl][:, c0 : c0 + N], gtiles["o"], tch)

        ol = DBG_MIN_LVL
        h_fin = h_t[ol][:, :BL]
        c_fin = c_t[ol][:, :BL] if ol in c_t else h_t[ol][:, :BL]
        eng = nc.sync if ol == 0 else nc.gpsimd
        eng.dma_start(out=h_out_d[:, :], in_=h_fin)
        eng.dma_start(out=c_out_d[:, :], in_=c_fin)

    nc.finalize()
    _CACHE["nc"] = nc
    return nc


def _wrap_idx(seg):
    """dma_gather index layout: unwrapped[i] = idxs[i % 16, i // 16],
    replicated across the 128 partitions."""
    w = seg.reshape(-1, 16).T.astype(np.int16)
    return np.tile(w, (8, 1))


def prep_core_inputs(tokens_c, arity_c, shared):
    """Per-core input map. tokens_c [BL,511], arity_c [BL,255]."""
    tokens_c = np.asarray(tokens_c)
    arity_c = np.asarray(arity_c)

    # per-level sigma-ordered tokens (padded to LVL_PW)
    tok_sig = {}
    for l in range(D + 1):
        off, cnt = 2**l - 1, 2**l
        flat = tokens_c[:, off : off + cnt].reshape(-1)[SIG[l]]
        pw = LVL_PW[l]
        if pw != flat.size:
            flat = np.concatenate([flat, np.zeros(pw - flat.size, np.int64)])
        tok_sig[l] = flat

    idx_cols = []
    for lvl, c0, width in GATHER_CALLS:
        idx_cols.append(_wrap_idx(tok_sig[lvl][c0 : c0 + width]))
    gidx = np.concatenate(idx_cols, axis=1)
    assert gidx.shape == (128, IDX_COLS)

    maskb = np.zeros((1, MASKB_LEN), BF16)
    for cid, lvl, c0, N, moff in CHUNKS:
        off = 2**lvl - 1
        m_flat = (arity_c[:, off : off + 2**lvl].reshape(-1) == 1).astype(
            np.float32
        )[SIG[lvl]]
        maskb[0, moff : moff + N] = m_flat[c0 : c0 + N].astype(BF16)

    return dict(
        shared,
        gidx=gidx,
        maskb=maskb,
        mbcast=np.broadcast_to(maskb, (128, MASKB_LEN)).copy(),
    )


def prep_shared_inputs(emb, W, bW, Ubin, bUbin, Uun, bUun):
    emb = np.asarray(emb, np.float32)
    W = np.asarray(W, np.float32)
    bW = np.asarray(bW, np.float32)
    Ubin = np.asarray(Ubin, np.float32)
    bUbin = np.asarray(bUbin, np.float32)
    Uun = np.asarray(Uun, np.float32)
    bUun = np.asarray(bUun, np.float32)

    biases = np.stack(
        [
            bW[3],                # leaf
            bW[0] + bUun[0],      # i common
            bW[1] + bUun[1],      # fL common
            bW[1] + bUbin[2] - 40.0,  # fR (binary-only; -40 kills unary)
            bW[2] + bUun[2],      # o common
            bW[3] + bUun[3],      # u common
        ]
    ).astype(np.float32)
    deltas = np.stack(
        [
            bUbin[0] - bUun[0],
            bUbin[1] - bUun[1],
            bUbin[3] - bUun[2],
            bUbin[4] - bUun[3],
            np.full(H, 40.0, np.float32),
        ]
    ).astype(BF16)

    ubt = Ubin[:, :H, :].copy()  # top half acts on h_l
    # Uun-folding: gates i,fl,o,u get (Ubt - Uun); fr keeps Ubt
    ubt_eff = ubt.copy()
    ubt_eff[0] -= Uun[0]
    ubt_eff[1] -= Uun[1]
    ubt_eff[3] -= Uun[2]
    ubt_eff[4] -= Uun[3]

    return dict(
        emb_bf=emb.astype(BF16),
        w_bf=W.astype(BF16),
        ubt_bf=ubt_eff.astype(BF16),
        ubb_bf=Ubin[:, H:, :].astype(BF16),
        uun_bf=Uun.astype(BF16),
        biases=biases,
        deltas=deltas,
    )


def kernel(tokens, arity, emb, W, bW, Ubin, bUbin, Uun, bUun):
    from concourse.bass_utils import run_bass_kernel_spmd

    tokens = np.asarray(tokens)
    arity = np.asarray(arity)

    shared = prep_shared_inputs(emb, W, bW, Ubin, bUbin, Uun, bUun)
    in_maps = [
        prep_core_inputs(
            tokens[k * BL : (k + 1) * BL], arity[k * BL : (k + 1) * BL], shared
        )
        for k in range(NCORES)
    ]

    nc = _build_nc()
    res = run_bass_kernel_spmd(nc, in_maps, core_ids=list(range(NCORES)))
    results = res.results

    h = np.concatenate([r["h_out"].T for r in results], axis=0)
    c = np.concatenate([r["c_out"].T for r in results], axis=0)
    return h.astype(np.float32), c.astype(np.float32)


# revision 9
# speedup vs baseline: 1.3937x; 1.0841x over previous
"""MixedArityTreeLSTM Trainium2 kernel (v2: pair-split layout).

Level-synchronous bottom-up Tree-LSTM over B=256 heap-indexed perfect binary
trees (511 nodes, depth 8), E=H=128. Pure data-parallel over 8 NeuronCores
(32 trees per core); all weights replicated.

Per-core layout: activations feature-major [H(part), nodes(free)]. Each
level is stored in "pair-split" order: for the parent level's column order
sigma_l, the child level is stored as [all left children in sigma_l order |
all right children in sigma_l order]. This makes every child operand a
CONTIGUOUS block (left block / right block), so vector ops run in 2x DVE
mode and no strided views are needed. The permutations are baked host-side
into the gather indices and masks; level 0 order is the identity.

Arity blending is folded into the matmuls via masked children + Uun-folding:
    pre_g = W_g^T x + Uun_g^T h_l + (Ubt_g - Uun_g)^T (m*h_l)
            + Ubb_g^T (m*h_r) + m*(b_bin_g - b_un_g) [K=1 matmul]
            + (bW_g + b_un_g) [ACT bias]
Matmul operands bf16; PSUM fp32 (leaf psum bf16); gates/h/c stored bf16.
Embedding gather uses gpsimd dma_gather(transpose=True) on a bf16 table.
"""

import os

import numpy as np
import ml_dtypes

DBG_MIN_LVL = int(os.environ.get("TL_MIN_LVL", "0"))
N_QUEUES = int(os.environ.get("TL_NQ", "2"))
C_FP32 = os.environ.get("TL_C_FP32", "") == "1"  # keep c/gates fp32 (debug)

B, D = 256, 8
V, E, H = 32000, 128, 128
N_NODES = 2 ** (D + 1) - 1  # 511
NCORES = 8
BL = B // NCORES  # 32 trees per core

LVL_N = {l: BL * (2**l) for l in range(D + 1)}
LVL_PW = {l: max(128, BL * (2**l)) for l in range(D + 1)}

# chunk widths: leaves at 1024, internal levels at 512
CHUNKW = {l: 512 for l in range(D + 1)}
CPL = {l: max(1, LVL_N[l] // CHUNKW[l]) for l in range(D + 1)}

# pair-split sigma orders (within-level tree-major flat indices)
SIG = {0: np.arange(BL, dtype=np.int64)}
for _l in range(0, D):
    _f = SIG[_l]
    _t, _p = _f // (2**_l), _f % (2**_l)
    _le = _t * (2 ** (_l + 1)) + 2 * _p
    SIG[_l + 1] = np.concatenate([_le, _le + 1])


def _child_chunks(l, j):
    """Child-level chunk ids needed by compute chunk (l, j)."""
    c0 = j * CHUNKW[l]
    N = min(CHUNKW[l], LVL_N[l] - c0)
    half = LVL_N[l]
    cw = CHUNKW[l + 1]
    s = set()
    for a, b in ((c0, c0 + N), (half + c0, half + c0 + N)):
        for k in range(a // cw, (b - 1) // cw + 1):
            s.add(k)
    return sorted(s)


ORDER = []
_emitted = set()


def _post(l, j):
    if (l, j) in _emitted:
        return
    if l < D:
        for k in _child_chunks(l, j):
            _post(l + 1, k)
    _emitted.add((l, j))
    ORDER.append((l, j))


_post(0, 0)

# gather calls, one per chunk in wave order: (lvl, col0, width in padded xT)
GATHER_CALLS = [
    (lvl, j * CHUNKW[lvl], min(CHUNKW[lvl], LVL_PW[lvl] - j * CHUNKW[lvl]))
    for lvl, j in ORDER
]

# internal-level compute chunks in wave order: (cid, lvl, c0, N, mask offset)
CHUNKS = []
_moff = 0
for lvl, j in ORDER:
    if lvl == D:
        continue
    N = min(CHUNKW[lvl], LVL_N[lvl] - j * CHUNKW[lvl])
    CHUNKS.append((len(CHUNKS), lvl, j * CHUNKW[lvl], N, _moff))
    _moff += N
MASKB_LEN = _moff  # 8160

IDX_COLS = sum(w // 16 for _, _, w in GATHER_CALLS)

BF16 = ml_dtypes.bfloat16

_CACHE = {}


def _build_nc():
    if "nc" in _CACHE:
        return _CACHE["nc"]

    from contextlib import ExitStack

    import concourse.mybir as mybir
    import concourse.tile as tile
    from concourse import bacc

    dt = mybir.dt
    AF = mybir.ActivationFunctionType
    gdt = dt.float32 if C_FP32 else dt.bfloat16

    nc = bacc.Bacc(num_swdge_queues=N_QUEUES)

    emb_d = nc.dram_tensor("emb_bf", [V, E], dt.bfloat16, kind="ExternalInput")
    idx_d = nc.dram_tensor("gidx", [128, IDX_COLS], dt.int16, kind="ExternalInput")
    mbc_d = nc.dram_tensor(
        "mbcast", [128, MASKB_LEN], dt.bfloat16, kind="ExternalInput"
    )
    maskb_d = nc.dram_tensor(
        "maskb", [1, MASKB_LEN], dt.bfloat16, kind="ExternalInput"
    )
    w_d = nc.dram_tensor("w_bf", [4, E, H], dt.bfloat16, kind="ExternalInput")
    # ubt_eff rows: i,fl: Ubt-Uun; fr: Ubt; o,u: Ubt-Uun (see prep)
    ubt_d = nc.dram_tensor("ubt_bf", [5, H, H], dt.bfloat16, kind="ExternalInput")
    ubb_d = nc.dram_tensor("ubb_bf", [5, H, H], dt.bfloat16, kind="ExternalInput")
    uun_d = nc.dram_tensor("uun_bf", [4, H, H], dt.bfloat16, kind="ExternalInput")
    # bias rows: 0=b_leaf 1=bc_i 2=bc_fL 3=b_fR 4=bc_o 5=bc_u
    bias_d = nc.dram_tensor("biases", [6, H], dt.float32, kind="ExternalInput")
    # delta rows: 0=d_i 1=d_fL 2=d_o 3=d_u 4=+40 (f_r unary kill)
    delt_d = nc.dram_tensor("deltas", [5, H], dt.bfloat16, kind="ExternalInput")

    h_out_d = nc.dram_tensor("h_out", [H, BL], dt.float32, kind="ExternalOutput")
    c_out_d = nc.dram_tensor("c_out", [H, BL], dt.float32, kind="ExternalOutput")

    with tile.TileContext(nc) as tc, ExitStack() as ctx:
        consts = ctx.enter_context(tc.tile_pool(name="consts", bufs=1))

        idx_sb = consts.tile([128, IDX_COLS], dt.int16)
        nc.sync.dma_start(out=idx_sb, in_=idx_d[:, :])
        w_sb = consts.tile([E, 4, H], dt.bfloat16)
        nc.sync.dma_start(out=w_sb, in_=w_d[:, :, :].rearrange("g e h -> e g h"))
        ubt_sb = consts.tile([H, 5, H], dt.bfloat16)
        nc.sync.dma_start(out=ubt_sb, in_=ubt_d[:, :, :].rearrange("g k h -> k g h"))
        ubb_sb = consts.tile([H, 5, H], dt.bfloat16)
        nc.sync.dma_start(out=ubb_sb, in_=ubb_d[:, :, :].rearrange("g k h -> k g h"))
        uun_sb = consts.tile([H, 4, H], dt.bfloat16)
        nc.sync.dma_start(out=uun_sb, in_=uun_d[:, :, :].rearrange("g k h -> k g h"))
        bias_sb = consts.tile([H, 6], dt.float32)
        nc.sync.dma_start(out=bias_sb, in_=bias_d[:, :].rearrange("n h -> h n"))
        delt_sb = consts.tile([1, 5, H], dt.bfloat16)
        nc.sync.dma_start(
            out=delt_sb, in_=delt_d[:, :].rearrange("(o g) h -> o g h", o=1)
        )
        mbc_sb = consts.tile([128, MASKB_LEN], dt.bfloat16)
        nc.sync.dma_start(out=mbc_sb, in_=mbc_d[:, :])
        maskb_sb = consts.tile([1, MASKB_LEN], dt.bfloat16)
        nc.sync.dma_start(out=maskb_sb, in_=maskb_d[:, :])

        # per-level xT tiles
        lev = ctx.enter_context(tc.tile_pool(name="lev", bufs=1))
        xt = {}
        for lvl in range(D, -1, -1):
            xt[lvl] = lev.tile(
                [128, LVL_PW[lvl]], dt.bfloat16, name=f"xTl{lvl}", tag=f"xTl{lvl}"
            )

        psum = ctx.enter_context(tc.tile_pool(name="psum", bufs=8, space="PSUM"))
        work = ctx.enter_context(tc.tile_pool(name="work", bufs=2))

        h_t = {}
        c_t = {}
        h_t[D] = lev.tile([H, LVL_N[D]], dt.bfloat16, name="h_leaf", tag="h_leaf")

        icols = {}
        _ic = 0
        for gi_, (lvl, c0, width) in enumerate(GATHER_CALLS):
            icols[(lvl, c0)] = (_ic, width, gi_)
            _ic += width // 16

        cid_of = {(lvl, c0): (cid, N, moff) for cid, lvl, c0, N, moff in CHUNKS}

        # gate -> (W idx, ubt_eff idx, uun idx or None, delta idx, bias col)
        GATES_FULL = [
            ("i", 0, 0, 0, 0, 1),
            ("fl", 1, 1, 1, 1, 2),
            ("fr", 1, 2, None, 4, 3),
            ("o", 2, 3, 2, 2, 4),
            ("u", 3, 4, 3, 3, 5),
        ]
        GATES_TOP = [GATES_FULL[0], GATES_FULL[3], GATES_FULL[4]]

        for lvl, j in ORDER:
            g0 = j * CHUNKW[lvl]
            _icol, width, gi_ = icols[(lvl, g0)]
            out_view = xt[lvl][:, g0 : g0 + width].rearrange(
                "p (o n) -> p o n", o=1
            )
            nc.gpsimd.dma_gather(
                out_view,
                emb_d[:, :],
                idx_sb[:, _icol : _icol + width // 16],
                width,
                width,
                E,
                transpose=True,
                queue_num=gi_ % N_QUEUES,
            )

            if lvl == D:
                for s0 in range(g0, g0 + width, 512):
                    sw = min(512, g0 + width - s0)
                    ps = psum.tile([H, sw], dt.float32, tag="pg", name="ps_leaf")
                    nc.tensor.matmul(
                        ps, w_sb[:, 3, :], xt[D][:, s0 : s0 + sw],
                        start=True, stop=True,
                    )
                    nc.scalar.activation(
                        h_t[D][:, s0 : s0 + sw], ps, AF.Tanh, bias=bias_sb[:, 0:1]
                    )
                continue

            cid, N, moff = cid_of[(lvl, g0)]
            c0 = g0
            if lvl < DBG_MIN_LVL:
                continue
            if c0 == 0:
                n = LVL_N[lvl]
                hdt = dt.float32 if lvl == 0 else dt.bfloat16
                cdt = dt.float32 if lvl == 0 else gdt
                h_t[lvl] = lev.tile([H, n], hdt, name=f"h_l{lvl}", tag=f"h_l{lvl}")
                c_t[lvl] = lev.tile([H, n], cdt, name=f"c_l{lvl}", tag=f"c_l{lvl}")

            half = LVL_N[lvl]
            hch = h_t[lvl + 1]
            h_l = hch[:, c0 : c0 + N]
            h_r = hch[:, half + c0 : half + c0 + N]

            mb = mbc_sb[:, moff : moff + N]
            hlb = work.tile([128, N], dt.bfloat16, tag="hlb", name="hlb")
            nc.vector.tensor_mul(hlb, h_l, mb)
            hrb = work.tile([128, N], dt.bfloat16, tag="hrb", name="hrb")
            nc.vector.tensor_mul(hrb, h_r, mb)

            xs = xt[lvl][:, c0 : c0 + N]
            mrow = maskb_sb[:, moff : moff + N]
            top = lvl == D - 1
            gates = GATES_TOP if top else GATES_FULL

            pts = {}
            for gname, wi, ubi, uui, di, _bi in gates:
                ps = psum.tile([H, N], dt.float32, tag="pg", name=f"ps_{gname}")
                # child-independent matmuls first: PE can start while DVE
                # still produces hlb/hrb
                nc.tensor.matmul(
                    ps, delt_sb[:, di, :], mrow, start=True, stop=False
                )
                nc.tensor.matmul(ps, w_sb[:, wi, :], xs, start=False, stop=False)
                if uui is not None:
                    nc.tensor.matmul(
                        ps, uun_sb[:, uui, :], h_l, start=False, stop=False
                    )
                nc.tensor.matmul(ps, ubt_sb[:, ubi, :], hlb, start=False, stop=False)
                nc.tensor.matmul(
                    ps, ubb_sb[:, ubi, :], hrb, start=False, stop=True
                )
                pts[gname] = ps

            gtiles = {}
            for gname, _wi, _ubi, _uui, _di, bi in gates:
                g = work.tile([128, N], gdt, tag=f"g{gname}", name=f"g{gname}")
                fn = AF.Tanh if gname == "u" else AF.Sigmoid
                nc.scalar.activation(g, pts[gname], fn, bias=bias_sb[:, bi : bi + 1])
                gtiles[gname] = g

            cs = c_t[lvl][:, c0 : c0 + N]
            if top:
                nc.vector.tensor_mul(cs, gtiles["i"], gtiles["u"])
            else:
                cch = c_t[lvl + 1]
                c_l = cch[:, c0 : c0 + N]
                c_r = cch[:, half + c0 : half + c0 + N]
                t1 = work.tile([128, N], gdt, tag="t1", name="t1")
                nc.vector.tensor_mul(t1, gtiles["i"], gtiles["u"])
                f2l = work.tile([128, N], gdt, tag="f2l", name="f2l")
                nc.vector.tensor_mul(f2l, gtiles["fl"], c_l)
                f2r = work.tile([128, N], gdt, tag="f2r", name="f2r")
                nc.vector.tensor_mul(f2r, gtiles["fr"], c_r)
                fs = work.tile([128, N], gdt, tag="fs", name="fs")
                nc.vector.tensor_add(fs, f2l, f2r)
                nc.vector.tensor_add(cs, t1, fs)

            tch = work.tile([128, N], gdt, tag="tch", name="tch")
            nc.scalar.activation(tch, cs, AF.Tanh)
            nc.vector.tensor_mul(h_t[lvl][:, c0 : c0 + N], gtiles["o"], tch)

        ol = DBG_MIN_LVL
        h_fin = h_t[ol][:, :BL]
        c_fin = c_t[ol][:, :BL] if ol in c_t else h_t[ol][:, :BL]
        eng = nc.sync if ol == 0 else nc.gpsimd
        eng.dma_start(out=h_out_d[:, :], in_=h_fin)
        eng.dma_start(out=c_out_d[:, :], in_=c_fin)

    nc.finalize()
    _CACHE["nc"] = nc
    return nc


def _wrap_idx(seg):
    """dma_gather index layout: unwrapped[i] = idxs[i % 16, i // 16],
    replicated across the 128 partitions."""
    w = seg.reshape(-1, 16).T.astype(np.int16)
    return np.tile(w, (8, 1))


def prep_core_inputs(tokens_c, arity_c, shared):
    """Per-core input map. tokens_c [BL,511], arity_c [BL,255]."""
    tokens_c = np.asarray(tokens_c)
    arity_c = np.asarray(arity_c)

    # per-level sigma-ordered tokens (padded to LVL_PW)
    tok_sig = {}
    for l in range(D + 1):
        off, cnt = 2**l - 1, 2**l
        flat = tokens_c[:, off : off + cnt].reshape(-1)[SIG[l]]
        pw = LVL_PW[l]
        if pw != flat.size:
            flat = np.concatenate([flat, np.zeros(pw - flat.size, np.int64)])
        tok_sig[l] = flat

    idx_cols = []
    for lvl, c0, width in GATHER_CALLS:
        idx_cols.append(_wrap_idx(tok_sig[lvl][c0 : c0 + width]))
    gidx = np.concatenate(idx_cols, axis=1)
    assert gidx.shape == (128, IDX_COLS)

    maskb = np.zeros((1, MASKB_LEN), BF16)
    for cid, lvl, c0, N, moff in CHUNKS:
        off = 2**lvl - 1
        m_flat = (arity_c[:, off : off + 2**lvl].reshape(-1) == 1).astype(
            np.float32
        )[SIG[lvl]]
        maskb[0, moff : moff + N] = m_flat[c0 : c0 + N].astype(BF16)

    return dict(
        shared,
        gidx=gidx,
        maskb=maskb,
        mbcast=np.broadcast_to(maskb, (128, MASKB_LEN)).copy(),
    )


def prep_shared_inputs(emb, W, bW, Ubin, bUbin, Uun, bUun):
    emb = np.asarray(emb, np.float32)
    W = np.asarray(W, np.float32)
    bW = np.asarray(bW, np.float32)
    Ubin = np.asarray(Ubin, np.float32)
    bUbin = np.asarray(bUbin, np.float32)
    Uun = np.asarray(Uun, np.float32)
    bUun = np.asarray(bUun, np.float32)

    biases = np.stack(
        [
            bW[3],                # leaf
            bW[0] + bUun[0],      # i common
            bW[1] + bUun[1],      # fL common
            bW[1] + bUbin[2] - 40.0,  # fR (binary-only; -40 kills unary)
            bW[2] + bUun[2],      # o common
            bW[3] + bUun[3],      # u common
        ]
    ).astype(np.float32)
    deltas = np.stack(
        [
            bUbin[0] - bUun[0],
            bUbin[1] - bUun[1],
            bUbin[3] - bUun[2],
            bUbin[4] - bUun[3],
            np.full(H, 40.0, np.float32),
        ]
    ).astype(BF16)

    ubt = Ubin[:, :H, :].copy()  # top half acts on h_l
    # Uun-folding: gates i,fl,o,u get (Ubt - Uun); fr keeps Ubt
    ubt_eff = ubt.copy()
    ubt_eff[0] -= Uun[0]
    ubt_eff[1] -= Uun[1]
    ubt_eff[3] -= Uun[2]
    ubt_eff[4] -= Uun[3]

    return dict(
        emb_bf=emb.astype(BF16),
        w_bf=W.astype(BF16),
        ubt_bf=ubt_eff.astype(BF16),
        ubb_bf=Ubin[:, H:, :].astype(BF16),
        uun_bf=Uun.astype(BF16),
        biases=biases,
        deltas=deltas,
    )


def kernel(tokens, arity, emb, W, bW, Ubin, bUbin, Uun, bUun):
    from concourse.bass_utils import run_bass_kernel_spmd

    tokens = np.asarray(tokens)
    arity = np.asarray(arity)

    shared = prep_shared_inputs(emb, W, bW, Ubin, bUbin, Uun, bUun)
    in_maps = [
        prep_core_inputs(
            tokens[k * BL : (k + 1) * BL], arity[k * BL : (k + 1) * BL], shared
        )
        for k in range(NCORES)
    ]

    nc = _build_nc()
    res = run_bass_kernel_spmd(nc, in_maps, core_ids=list(range(NCORES)))
    results = res.results

    h = np.concatenate([r["h_out"].T for r in results], axis=0)
    c = np.concatenate([r["c_out"].T for r in results], axis=0)
    return h.astype(np.float32), c.astype(np.float32)
